# revision 1
# baseline (speedup 1.0000x reference)
"""Trainium2 Bass kernel for the LSTM classifier problem.

Strategy (data parallel over 8 NeuronCores, batch 2048 -> 256/core):
  - All four gates computed as tanh() only (sigmoid(z) = (tanh(z/2)+1)/2 with
    the 1/2 folded into the weights), so each timestep needs exactly two
    activation instructions over the [128, B] gate tiles plus one tanh(c).
  - h is stored doubled (h2 = 2h = (tau_o+1)*tanh(c)); the 0.5 compensation is
    folded into the W_hh columns and fc_W.
  - Per step:  PE: x-proj + h-proj matmuls (fp32r, N=256 -> 1 cycle/row) into
    one [128, 512] PSUM bank holding [f;i | o;g] pre-activations;
    ACT: tanh over each half (bias rides the activation instruction);
    DVE: two scalar_tensor_tensor ops (tau+1)*other for i*g and f*c, one for h;
    PE: stacked-0.5-identity matmul adds the partition-split products into c.
  - x is host-transposed to [T, D, B] so the per-chunk DMA is dense.
"""

import math
import os
import numpy as np

import concourse.bass as bass
import concourse.bacc as bacc
import concourse.mybir as mybir
import concourse.tile as tile
from concourse.bass_utils import run_bass_kernel_spmd

F32 = mybir.dt.float32
F32R = mybir.dt.float32r
ADD = mybir.AluOpType.add
MULT = mybir.AluOpType.mult
TANH = mybir.ActivationFunctionType.Tanh
IDENT = mybir.ActivationFunctionType.Identity

H = 64
D = 32
C_OUT = 10
N_CORES = 8


def build_lstm_nc(T: int, Bc: int, xs_steps: int = 8, trace_label: str = "lstm"):
    """Build the per-core Bass module. Bc = batch per core."""
    nc = bacc.Bacc("TRN2", target_bir_lowering=False, debug=False,
                   num_devices=N_CORES)

    xT = nc.dram_tensor("xT", [T, D, Bc], F32R, kind="ExternalInput")
    w_ih = nc.dram_tensor("w_ih", [2, D, 128], F32R, kind="ExternalInput")
    w_hh = nc.dram_tensor("w_hh", [2, H, 128], F32R, kind="ExternalInput")
    biases = nc.dram_tensor("biases", [2, 128, 1], F32, kind="ExternalInput")
    fc_w = nc.dram_tensor("fc_w", [H, C_OUT], F32R, kind="ExternalInput")
    fc_b = nc.dram_tensor("fc_b", [C_OUT, 1], F32, kind="ExternalInput")
    out = nc.dram_tensor("out", [C_OUT, Bc], F32, kind="ExternalOutput")

    n_chunks = T // xs_steps
    assert T % xs_steps == 0

    with tile.TileContext(nc) as tc:
        with (
            tc.tile_pool(name="consts", bufs=1) as consts,
            tc.tile_pool(name="xs", bufs=4) as xs_pool,
            tc.tile_pool(name="taus", bufs=2) as tau_pool,
            tc.tile_pool(name="u", bufs=2) as u_pool,
            tc.tile_pool(name="tc3", bufs=2) as tc3_pool,
            tc.tile_pool(name="h", bufs=3) as h_pool,
            tc.tile_pool(name="gpsum", bufs=2, space="PSUM") as gpsum_pool,
            tc.tile_pool(name="cpsum", bufs=2, space="PSUM") as cpsum_pool,
            tc.tile_pool(name="fcpsum", bufs=1, space="PSUM") as fc_pool,
        ):
            # ---- constants into SBUF ----
            wih_sb = consts.tile([D, 2 * 128], F32R)    # [:, 0:128]=FI, [:,128:256]=OG
            whh_sb = consts.tile([H, 2 * 128], F32R)
            bias_sb = consts.tile([128, 2], F32)       # col 0 = FI bias, col 1 = OG
            ist_sb = consts.tile([128, H], F32R)        # stacked 0.5*I
            fcw_sb = consts.tile([H, C_OUT], F32R)
            fcb_sb = consts.tile([C_OUT, 1], F32)
            nc.sync.dma_start(out=wih_sb[:, 0:128], in_=w_ih[0])
            nc.sync.dma_start(out=wih_sb[:, 128:256], in_=w_ih[1])
            nc.sync.dma_start(out=whh_sb[:, 0:128], in_=w_hh[0])
            nc.sync.dma_start(out=whh_sb[:, 128:256], in_=w_hh[1])
            nc.sync.dma_start(out=bias_sb[:, 0:1], in_=biases[0])
            nc.sync.dma_start(out=bias_sb[:, 1:2], in_=biases[1])
            nc.sync.dma_start(out=fcw_sb[:], in_=fc_w[:])
            nc.sync.dma_start(out=fcb_sb[:], in_=fc_b[:])

            # stacked halved identity for the cross-partition add (built on
            # device: iota-based would need gpsimd; cheaper to DMA it in).
            ist_dram = nc.dram_tensor("istack", [128, H], F32R,
                                      kind="ExternalInput")
            nc.sync.dma_start(out=ist_sb[:], in_=ist_dram[:])

            # ---- state init ----
            h2 = h_pool.tile([H, Bc], F32R)
            nc.vector.memset(h2[:].bitcast(mybir.dt.uint32), 0)
            c_prev = cpsum_pool.tile([H, Bc], F32)
            nc.vector.memset(c_prev[:], 0.0)

            for chunk in range(n_chunks):
                xs = xs_pool.tile([D, xs_steps * Bc], F32R)
                nc.sync.dma_start(
                    out=xs[:].rearrange("d (t b) -> d t b", t=xs_steps),
                    in_=xT[chunk * xs_steps:(chunk + 1) * xs_steps]
                    .rearrange("t d b -> d t b"),
                )
                for s in range(xs_steps):
                    t = chunk * xs_steps + s
                    x_t = xs[:, s * Bc:(s + 1) * Bc]

                    gp = gpsum_pool.tile([128, 2 * Bc], F32)
                    # FI half
                    nc.tensor.matmul(gp[:, 0:Bc], wih_sb[:, 0:128],
                                     x_t, start=True, stop=False)
                    nc.tensor.matmul(gp[:, 0:Bc], whh_sb[:, 0:128],
                                     h2[:], start=False, stop=True)
                    # OG half
                    nc.tensor.matmul(gp[:, Bc:2 * Bc], wih_sb[:, 128:256],
                                     x_t, start=True, stop=False)
                    nc.tensor.matmul(gp[:, Bc:2 * Bc], whh_sb[:, 128:256],
                                     h2[:], start=False, stop=True)

                    tau_fi = tau_pool.tile([128, Bc], F32)
                    tau_og = tau_pool.tile([128, Bc], F32, tag="tau_og")
                    # rows 0:64 = tau_f, 64:128 = tau_i
                    nc.scalar.activation(tau_fi[:], gp[:, 0:Bc], TANH,
                                         bias=bias_sb[:, 0:1])
                    # rows 0:64 = tau_o, 64:128 = tau_g
                    nc.scalar.activation(tau_og[:], gp[:, Bc:2 * Bc], TANH,
                                         bias=bias_sb[:, 1:2])

                    u = u_pool.tile([128, Bc], F32R)
                    # U_lo = (tau_f + 1) * c  (= 2 f c)
                    nc.vector.scalar_tensor_tensor(
                        u[0:H], tau_fi[0:H], 1.0, c_prev[:], ADD, MULT)
                    # U_hi = (tau_i + 1) * tau_g  (= 2 i g)
                    nc.vector.scalar_tensor_tensor(
                        u[H:128], tau_fi[H:128], 1.0, tau_og[H:128],
                        ADD, MULT)

                    c_new = cpsum_pool.tile([H, Bc], F32)
                    nc.tensor.matmul(c_new[:], ist_sb[:], u[:],
                                     start=True, stop=True)

                    tc3 = tc3_pool.tile([H, Bc], F32)
                    nc.scalar.activation(tc3[:], c_new[:], TANH)

                    h2 = h_pool.tile([H, Bc], F32R)
                    # h2 = (tau_o + 1) * tanh(c)
                    nc.vector.scalar_tensor_tensor(
                        h2[:], tau_og[0:H], 1.0, tc3[:], ADD, MULT)

                    c_prev = c_new

            # ---- final FC: logits^T = (0.5 fc_W)^T-ish (host-prescaled) ----
            fcp = fc_pool.tile([C_OUT, Bc], F32)
            nc.tensor.matmul(fcp[:], fcw_sb[:], h2[:],
                             start=True, stop=True)
            logits_sb = consts.tile([C_OUT, Bc], F32)
            nc.scalar.activation(logits_sb[:], fcp[:], IDENT,
                                 bias=fcb_sb[:])
            nc.sync.dma_start(out=out[:], in_=logits_sb[:])

    nc.compile()
    return nc


def _prep_weights(W_ih, W_hh, b_ih, b_hh, fc_W):
    Hh = H
    idx = {g: np.arange(k * Hh, (k + 1) * Hh) for g, k in zip("ifgo", range(4))}
    rows_FI = np.concatenate([idx["f"], idx["i"]])
    rows_OG = np.concatenate([idx["o"], idx["g"]])
    s_FI = np.full(128, 0.5, np.float32)
    s_OG = np.concatenate([np.full(64, 0.5, np.float32),
                           np.full(64, 1.0, np.float32)])
    b_sum = (b_ih + b_hh).astype(np.float32)

    w_ih_arr = np.stack([
        (s_FI[:, None] * W_ih[rows_FI]).T,          # [D, 128]
        (s_OG[:, None] * W_ih[rows_OG]).T,
    ]).astype(np.float32)                            # [2, D, 128]
    w_hh_arr = np.stack([
        (s_FI[:, None] * W_hh[rows_FI] * 0.5).T,     # [H, 128]
        (s_OG[:, None] * W_hh[rows_OG] * 0.5).T,
    ]).astype(np.float32)
    biases_arr = np.stack([s_FI * b_sum[rows_FI],
                           s_OG * b_sum[rows_OG]]).astype(np.float32)[:, :, None]
    ist = np.zeros((128, Hh), np.float32)
    ist[np.arange(Hh), np.arange(Hh)] = 0.5
    ist[np.arange(Hh) + Hh, np.arange(Hh)] = 0.5
    fcw_arr = (0.5 * fc_W).T.astype(np.float32)      # [H, C]
    return w_ih_arr, w_hh_arr, biases_arr, ist, fcw_arr


_NC_CACHE = {}


def kernel(x, W_ih, W_hh, b_ih, b_hh, fc_W, fc_b, _trace=False):
    x = np.asarray(x, np.float32)
    B, T, Dd = x.shape
    assert Dd == D
    Bc = B // N_CORES

    w_ih_arr, w_hh_arr, biases_arr, ist, fcw_arr = _prep_weights(
        np.asarray(W_ih, np.float32), np.asarray(W_hh, np.float32),
        np.asarray(b_ih, np.float32), np.asarray(b_hh, np.float32),
        np.asarray(fc_W, np.float32))
    fcb_arr = np.asarray(fc_b, np.float32).reshape(C_OUT, 1)

    key = (T, Bc)
    if key not in _NC_CACHE:
        _NC_CACHE[key] = build_lstm_nc(T, Bc)
    nc = _NC_CACHE[key]

    in_maps = []
    for cc in range(N_CORES):
        xs = x[cc * Bc:(cc + 1) * Bc]                  # [Bc, T, D]
        xTc = np.ascontiguousarray(xs.transpose(1, 2, 0))  # [T, D, Bc]
        in_maps.append({
            "xT": xTc, "w_ih": w_ih_arr, "w_hh": w_hh_arr,
            "biases": biases_arr, "istack": ist,
            "fc_w": fcw_arr, "fc_b": fcb_arr,
        })

    res = run_bass_kernel_spmd(nc, in_maps, core_ids=list(range(N_CORES)),
                               trace=_trace)
    outs = [r["out"] for r in res.results]            # each [C, Bc]
    logits = np.concatenate([o.T for o in outs], axis=0).astype(np.float32)
    if _trace:
        kernel.last_results = res
    return logits



# revision 6
# speedup vs baseline: 1.2639x; 1.2639x over previous
"""Trainium2 Bass kernel for the LSTM classifier problem.

Data parallel over 8 NeuronCores (batch 2048 -> 256/core), with each core
running a 2-group software pipeline (128 batch cols per group) so the two
recurrence chains interleave on the engines.

All gates are computed in tanh form (sigmoid(z) = (tanh(z/2)+1)/2 with the
1/2 pre-folded into weights); h is stored doubled (h2 = 2h) with the
compensation folded into W_hh and fc_W. The moving operand of the gate
matmuls is a stacked [x_t; 1; h2_{t-1}] tile (97 rows), so each gate matmul
is a single start&stop PSUM write (no accumulation groups) and the bias
rides the ones-row. The per-step DVE h2 write lands directly into the next
step's moving slot.

Per step, per group g (W = 128 batch cols):
  PE : FI and OG gate matmuls, stationary [97, 128] fp16 -> [128, W] PSUM.
  ACT: one tanh over the group's [128, 2W] gate block -> tau fp16.
  DVE: u_lo = (tau_f+1)*c, u_hi = (tau_i+1)*tau_g -> u fp16.
  PE : stacked-0.5-identity matmul adds the partition-split halves -> c PSUM.
  ACT: tanh(c) -> tc3 fp16.
  DVE: h2 = (tau_o+1)*tc3 -> next step's moving slot (fp16).
The ist matmul of each group is emitted late (just before its tanh(c)) so it
cannot head-of-line block the other group's chain on the in-order PE queue.
"""

import numpy as np

import concourse.bass as bass
import concourse.bacc as bacc
import concourse.mybir as mybir
import concourse.tile as tile
from concourse.bass_utils import run_bass_kernel_spmd

F32 = mybir.dt.float32
F16 = mybir.dt.float16
ADD = mybir.AluOpType.add
MULT = mybir.AluOpType.mult
TANH = mybir.ActivationFunctionType.Tanh
IDENT = mybir.ActivationFunctionType.Identity

H = 64
D = 32
R = 128       # stacked moving rows: x(32) + ones(1) + pad + h2 at 64:128
H2OFF = 64
C_OUT = 10
N_CORES = 8
G = 2          # pipeline groups per core
S_CHUNK = 8    # timesteps per x DMA chunk


def build_lstm_nc(T: int, Bc: int, trace_label: str = "lstm"):
    """Build the per-core Bass module. Bc = batch per core."""
    W = Bc // G
    nc = bacc.Bacc("TRN2", target_bir_lowering=False, debug=False,
                   num_devices=N_CORES)

    xT = nc.dram_tensor("xT", [D + 1, T, Bc], F16, kind="ExternalInput")
    sxh = nc.dram_tensor("sxh", [2, R, 128], F16, kind="ExternalInput")
    ist = nc.dram_tensor("istack", [128, H], F16, kind="ExternalInput")
    fc_w = nc.dram_tensor("fc_w", [H, C_OUT], F16, kind="ExternalInput")
    fc_b = nc.dram_tensor("fc_b", [C_OUT, 1], F32, kind="ExternalInput")
    out = nc.dram_tensor("out", [C_OUT, Bc], F32, kind="ExternalOutput")

    S = S_CHUNK
    n_chunks = T // S
    assert T % S == 0

    with tile.TileContext(nc) as tc:
        with (
            tc.tile_pool(name="consts", bufs=1) as consts,
            tc.tile_pool(name="xs", bufs=3) as xs_pool,
            tc.tile_pool(name="tau", bufs=4) as tau_pool,
            tc.tile_pool(name="u", bufs=4) as u_pool,
            tc.tile_pool(name="tc3", bufs=4) as tc3_pool,
            tc.tile_pool(name="hf", bufs=1) as hf_pool,
            tc.tile_pool(name="gpsum", bufs=3, space="PSUM") as gpsum_pool,
            tc.tile_pool(name="cpsum", bufs=2, space="PSUM") as cpsum_pool,
        ):
            sxh_sb = consts.tile([R, 2 * 128], F16)
            ist_sb = consts.tile([128, H], F16)
            fcw_sb = consts.tile([H, C_OUT], F16)
            fcb_sb = consts.tile([C_OUT, 1], F32)
            nc.sync.dma_start(out=sxh_sb[:, 0:128], in_=sxh[0])
            nc.sync.dma_start(out=sxh_sb[:, 128:256], in_=sxh[1])
            nc.sync.dma_start(out=ist_sb[:], in_=ist[:])
            nc.sync.dma_start(out=fcw_sb[:], in_=fc_w[:])
            nc.sync.dma_start(out=fcb_sb[:], in_=fc_b[:])

            # moving tiles: [R, S*Bc]; rows 0:33 = [x;1] (DMA), 33:97 = h2
            tiles = {}

            def get_tile(k):
                if k not in tiles:
                    tiles[k] = xs_pool.tile([R, S * Bc], F16, tag="xs",
                                            bufs=3, name=f"xs{k}")
                    if k < 3:
                        # zero the pad rows once per ring slot (the DMA
                        # rewrites row 32 with ones; 33:64 stay zero forever)
                        nc.vector.memset(
                            tiles[k][32:H2OFF, :].bitcast(mybir.dt.uint16), 0)
                    nc.sync.dma_start(
                        out=tiles[k][0:D + 1, :]
                        .rearrange("d (t b) -> d t b", t=S),
                        in_=xT[:, k * S:(k + 1) * S, :])
                return tiles[k]

            x0 = get_tile(0)
            nc.vector.memset(x0[H2OFF:R, 0:Bc].bitcast(mybir.dt.uint16), 0)

            cc = []
            for g in range(G):
                cg = cpsum_pool.tile([H, W], F32, tag=f"c{g}", bufs=2,
                                     name="cg")
                nc.vector.memset(cg[:], 0.0)
                cc.append(cg)

            h_fin = hf_pool.tile([H, Bc], F16)  # last step's h2 (for fc)

            pend = [None] * G      # (cc_tile, tau_tile, t) -> tanhc+h2
            pend_ist = [None] * G  # (u_tile, tau_tile, t) -> ist

            def emit_ist(g):
                u, tau_t, t = pend_ist[g]
                cn = cpsum_pool.tile([H, W], F32, tag=f"c{g}", bufs=2,
                                     name="cn")
                nc.tensor.matmul(cn[:], ist_sb[:], u[:], start=True, stop=True)
                cc[g] = cn
                pend[g] = (cn, tau_t, t)
                pend_ist[g] = None

            def phase2(g):
                cin, tau_t, t = pend[g]
                tc3 = tc3_pool.tile([H, W], F16, tag=f"tc{g}")
                nc.scalar.activation(tc3[:], cin[:], TANH)
                if t + 1 < T:
                    nxt = get_tile((t + 1) // S)
                    s2 = (t + 1) % S
                    hdst = nxt[H2OFF:R,
                               s2 * Bc + g * W:s2 * Bc + (g + 1) * W]
                else:
                    hdst = h_fin[:, g * W:(g + 1) * W]
                nc.vector.scalar_tensor_tensor(
                    hdst, tau_t[0:H, W:2 * W], 1.0, tc3[:], ADD, MULT)
                pend[g] = None

            def phase1(g, gp, xs, s, t):
                c0 = g * W
                mv = xs[:, s * Bc + c0:s * Bc + c0 + W]
                nc.tensor.matmul(gp[:, c0:c0 + W], sxh_sb[:, 0:128],
                                 mv, start=True, stop=True)
                nc.tensor.matmul(gp[:, Bc + c0:Bc + c0 + W],
                                 sxh_sb[:, 128:256], mv,
                                 start=True, stop=True)
                tau_t = tau_pool.tile([128, 2 * W], F16, tag=f"tau{g}")
                gin = gp[:].rearrange("p (h b) -> p h b", h=2)[:, :, c0:c0 + W]
                nc.scalar.activation(
                    tau_t[:].rearrange("p (h b) -> p h b", h=2), gin, TANH)
                u = u_pool.tile([128, W], F16, tag=f"u{g}")
                nc.vector.scalar_tensor_tensor(
                    u[0:H], tau_t[0:H, 0:W], 1.0, cc[g][:], ADD, MULT)
                nc.vector.scalar_tensor_tensor(
                    u[H:128], tau_t[H:128, 0:W], 1.0,
                    tau_t[H:128, W:2 * W], ADD, MULT)
                pend_ist[g] = (u, tau_t, t)

            for chunk in range(n_chunks):
                xs = get_tile(chunk)
                for s in range(S):
                    t = chunk * S + s
                    gp = gpsum_pool.tile([128, 2 * Bc], F32, tag="gp")
                    # pipeline: ACT order gates(0), tanhc(1|t-1), tanhc(0),
                    # gates(1); each group's ist lands just before its tanhc.
                    phase1(0, gp, xs, s, t)
                    if pend_ist[1] is not None:
                        emit_ist(1)
                    if pend[1] is not None:
                        phase2(1)
                    emit_ist(0)
                    phase2(0)
                    phase1(1, gp, xs, s, t)
            emit_ist(1)
            phase2(1)

            fcp = gpsum_pool.tile([C_OUT, Bc], F32, tag="fcp", bufs=1)
            nc.tensor.matmul(fcp[:], fcw_sb[:], h_fin[:],
                             start=True, stop=True)
            logits = consts.tile([C_OUT, Bc], F32)
            nc.scalar.activation(logits[:], fcp[:], IDENT, bias=fcb_sb[:])
            nc.sync.dma_start(out=out[:], in_=logits[:])

    nc.compile()
    return nc


def _prep_weights(W_ih, W_hh, b_ih, b_hh, fc_W):
    """Fold sigmoid->tanh halving, h2 doubling, and biases into stationaries.

    Gate order in the reference weights is (i, f, g, o). FI half = [f; i]
    with scale 0.5; OG half = [o; g] with scales (0.5, 1.0). Stationary
    rows: 0:32 x-weights, 32 bias, 64:128 h-weights (extra 0.5 for h2=2h).
    """
    idx = {g: np.arange(k * H, (k + 1) * H) for k, g in enumerate("ifgo")}
    rows_FI = np.concatenate([idx["f"], idx["i"]])
    rows_OG = np.concatenate([idx["o"], idx["g"]])
    s_FI = np.full(128, 0.5, np.float32)
    s_OG = np.concatenate([np.full(64, 0.5, np.float32),
                           np.full(64, 1.0, np.float32)])
    b_sum = (b_ih + b_hh).astype(np.float32)

    sxh = np.zeros((2, R, 128), np.float32)
    for k, (rows, sc) in enumerate([(rows_FI, s_FI), (rows_OG, s_OG)]):
        sxh[k, 0:D] = (sc[:, None] * W_ih[rows]).T
        sxh[k, D] = sc * b_sum[rows]
        sxh[k, H2OFF:R] = (sc[:, None] * W_hh[rows] * 0.5).T
    ist = np.zeros((128, H), np.float32)
    ist[np.arange(H), np.arange(H)] = 0.5
    ist[np.arange(H) + H, np.arange(H)] = 0.5
    fcw = (0.5 * fc_W).T
    return (sxh.astype(np.float16), ist.astype(np.float16),
            fcw.astype(np.float16))


_NC_CACHE = {}


def kernel(x, W_ih, W_hh, b_ih, b_hh, fc_W, fc_b, _trace=False):
    x = np.asarray(x, np.float32)
    B, T, Dd = x.shape
    assert Dd == D
    Bc = B // N_CORES

    sxh, ist, fcw = _prep_weights(
        np.asarray(W_ih, np.float32), np.asarray(W_hh, np.float32),
        np.asarray(b_ih, np.float32), np.asarray(b_hh, np.float32),
        np.asarray(fc_W, np.float32))
    fcb = np.asarray(fc_b, np.float32).reshape(C_OUT, 1)

    key = (T, Bc)
    if key not in _NC_CACHE:
        _NC_CACHE[key] = build_lstm_nc(T, Bc)
    nc = _NC_CACHE[key]

    in_maps = []
    for core in range(N_CORES):
        xsl = x[core * Bc:(core + 1) * Bc]            # [Bc, T, D]
        xTc = np.empty((D + 1, T, Bc), np.float16)
        xTc[0:D] = xsl.transpose(2, 1, 0).astype(np.float16)
        xTc[D] = 1.0
        in_maps.append({
            "xT": xTc, "sxh": sxh, "istack": ist,
            "fc_w": fcw, "fc_b": fcb,
        })

    res = run_bass_kernel_spmd(nc, in_maps, core_ids=list(range(N_CORES)),
                               trace=_trace)
    outs = [r["out"] for r in res.results]            # each [C, Bc]
    logits = np.concatenate([o.T for o in outs], axis=0).astype(np.float32)
    if _trace:
        kernel.last_results = res
    return logits


# revision 9
# speedup vs baseline: 1.4452x; 1.1434x over previous
"""Trainium2 Bass kernel for the LSTM classifier problem.

Data parallel over 8 NeuronCores (batch 2048 -> 256/core), with each core
running a 2-group software pipeline (128 batch cols per group) so the two
recurrence chains interleave on the engines.

All gates are computed in tanh form (sigmoid(z) = (tanh(z/2)+1)/2 with the
1/2 pre-folded into weights); h is stored doubled (h2 = 2h) with the
compensation folded into W_hh and fc_W. The moving operand of the gate
matmuls is a stacked [x_t; 1; h2_{t-1}] tile (97 rows), so each gate matmul
is a single start&stop PSUM write (no accumulation groups) and the bias
rides the ones-row. The per-step DVE h2 write lands directly into the next
step's moving slot.

Per step, per group g (W = 128 batch cols):
  PE : FI and OG gate matmuls, stationary [97, 128] fp16 -> [128, W] PSUM.
  ACT: one tanh over the group's [128, 2W] gate block -> tau fp16.
  DVE: u_lo = (tau_f+1)*c, u_hi = (tau_i+1)*tau_g -> u fp16.
  PE : stacked-0.5-identity matmul adds the partition-split halves -> c PSUM.
  ACT: tanh(c) -> tc3 fp16.
  DVE: h2 = (tau_o+1)*tc3 -> next step's moving slot (fp16).
The ist matmul of each group is emitted late (just before its tanh(c)) so it
cannot head-of-line block the other group's chain on the in-order PE queue.
"""

import numpy as np

import concourse.bass as bass
import concourse.bacc as bacc
import concourse.mybir as mybir
import concourse.tile as tile
from concourse.bass_utils import run_bass_kernel_spmd

F32 = mybir.dt.float32
F16 = mybir.dt.float16
ADD = mybir.AluOpType.add
MULT = mybir.AluOpType.mult
TANH = mybir.ActivationFunctionType.Tanh
IDENT = mybir.ActivationFunctionType.Identity

H = 64
D = 32
R = 128       # stacked moving rows: x(32) + ones(1) + pad + h2 at 64:128
H2OFF = 64
C_OUT = 10
N_CORES = 8
G = 4          # pipeline groups per core
S_CHUNK = 8    # timesteps per x DMA chunk


def build_lstm_nc(T: int, Bc: int, trace_label: str = "lstm",
                  tau_bufs=4, u_bufs=4, tc3_bufs=4, xs_bufs=3, gp_bufs=3):
    """Build the per-core Bass module. Bc = batch per core."""
    W = Bc // G
    nc = bacc.Bacc("TRN2", target_bir_lowering=False, debug=False,
                   num_devices=N_CORES)

    xT = nc.dram_tensor("xT", [D + 1, T, Bc], F16, kind="ExternalInput")
    sxh = nc.dram_tensor("sxh", [2, R, 128], F16, kind="ExternalInput")
    ist = nc.dram_tensor("istack", [128, H], F16, kind="ExternalInput")
    fc_w = nc.dram_tensor("fc_w", [H, C_OUT], F16, kind="ExternalInput")
    fc_b = nc.dram_tensor("fc_b", [C_OUT, 1], F32, kind="ExternalInput")
    out = nc.dram_tensor("out", [C_OUT, Bc], F32, kind="ExternalOutput")

    S = S_CHUNK
    n_chunks = T // S
    assert T % S == 0

    with tile.TileContext(nc) as tc:
        with (
            tc.tile_pool(name="consts", bufs=1) as consts,
            tc.tile_pool(name="xs", bufs=xs_bufs) as xs_pool,
            tc.tile_pool(name="tau", bufs=tau_bufs) as tau_pool,
            tc.tile_pool(name="u", bufs=u_bufs) as u_pool,
            tc.tile_pool(name="tc3", bufs=tc3_bufs) as tc3_pool,
            tc.tile_pool(name="hf", bufs=1) as hf_pool,
            tc.tile_pool(name="gpsum", bufs=gp_bufs, space="PSUM") as gpsum_pool,
            tc.tile_pool(name="cpsum", bufs=2, space="PSUM") as cpsum_pool,
        ):
            sxh_sb = consts.tile([R, 2 * 128], F16)
            ist_sb = consts.tile([128, H], F16)
            fcw_sb = consts.tile([H, C_OUT], F16)
            fcb_sb = consts.tile([C_OUT, 1], F32)
            nc.sync.dma_start(out=sxh_sb[:, 0:128], in_=sxh[0])
            nc.sync.dma_start(out=sxh_sb[:, 128:256], in_=sxh[1])
            nc.sync.dma_start(out=ist_sb[:], in_=ist[:])
            nc.sync.dma_start(out=fcw_sb[:], in_=fc_w[:])
            nc.sync.dma_start(out=fcb_sb[:], in_=fc_b[:])

            # moving tiles: [R, S*Bc]; rows 0:33 = [x;1] (DMA), 33:97 = h2
            tiles = {}

            def get_tile(k):
                if k not in tiles:
                    tiles[k] = xs_pool.tile([R, S * Bc], F16, tag="xs",
                                            bufs=xs_bufs, name=f"xs{k}")
                    if k < xs_bufs:
                        # zero the pad rows once per ring slot (the DMA
                        # rewrites row 32 with ones; 33:64 stay zero forever)
                        nc.vector.memset(
                            tiles[k][32:H2OFF, :].bitcast(mybir.dt.uint16), 0)
                    nc.sync.dma_start(
                        out=tiles[k][0:D + 1, :]
                        .rearrange("d (t b) -> d t b", t=S),
                        in_=xT[:, k * S:(k + 1) * S, :])
                return tiles[k]

            x0 = get_tile(0)
            nc.vector.memset(x0[H2OFF:R, 0:Bc].bitcast(mybir.dt.uint16), 0)

            cc = []
            if G <= 2:
                for g in range(G):
                    cg = cpsum_pool.tile([H, W], F32, tag=f"c{g}", bufs=2,
                                         name="cg")
                    nc.vector.memset(cg[:], 0.0)
                    cc.append(cg)
            else:
                # pair two groups per PSUM bank to fit 8 banks
                for p in range(G // 2):
                    cp = cpsum_pool.tile([H, 2 * W], F32, tag=f"cp{p}",
                                         bufs=2, name="cp")
                    nc.vector.memset(cp[:], 0.0)
                    cc.append(cp[:, 0:W])
                    cc.append(cp[:, W:2 * W])
            pair_cur = {}  # pair -> (step, tile) for G>2 ist allocation

            h_fin = hf_pool.tile([H, Bc], F16)  # last step's h2 (for fc)

            pend = [None] * G      # (cc_tile, tau_tile, t) -> tanhc+h2
            pend_ist = [None] * G  # (u_tile, tau_tile, t) -> ist

            def emit_ist(g):
                u, tau_t, t = pend_ist[g]
                if G <= 2:
                    cn = cpsum_pool.tile([H, W], F32, tag=f"c{g}", bufs=2,
                                         name="cn")[:]
                else:
                    p = g // 2
                    if pair_cur.get(p, (None,))[0] != t:
                        pair_cur[p] = (t, cpsum_pool.tile(
                            [H, 2 * W], F32, tag=f"cp{p}", bufs=2, name="cn"))
                    cn = pair_cur[p][1][:, (g % 2) * W:(g % 2 + 1) * W]
                nc.tensor.matmul(cn, ist_sb[:], u[:], start=True, stop=True)
                cc[g] = cn
                pend[g] = (cn, tau_t, t)
                pend_ist[g] = None

            def phase2(g):
                cin, tau_t, t = pend[g]
                tc3 = tc3_pool.tile([H, W], F16, tag=f"tc{g}")
                nc.scalar.activation(tc3[:], cin[:], TANH)
                if t + 1 < T:
                    nxt = get_tile((t + 1) // S)
                    s2 = (t + 1) % S
                    hdst = nxt[H2OFF:R,
                               s2 * Bc + g * W:s2 * Bc + (g + 1) * W]
                else:
                    hdst = h_fin[:, g * W:(g + 1) * W]
                nc.vector.scalar_tensor_tensor(
                    hdst, tau_t[0:H, W:2 * W], 1.0, tc3[:], ADD, MULT)
                pend[g] = None

            def phase1(g, gp, xs, s, t):
                c0 = g * W
                mv = xs[:, s * Bc + c0:s * Bc + c0 + W]
                nc.tensor.matmul(gp[:, c0:c0 + W], sxh_sb[:, 0:128],
                                 mv, start=True, stop=True)
                nc.tensor.matmul(gp[:, Bc + c0:Bc + c0 + W],
                                 sxh_sb[:, 128:256], mv,
                                 start=True, stop=True)
                tau_t = tau_pool.tile([128, 2 * W], F16, tag=f"tau{g}")
                gin = gp[:].rearrange("p (h b) -> p h b", h=2)[:, :, c0:c0 + W]
                nc.scalar.activation(
                    tau_t[:].rearrange("p (h b) -> p h b", h=2), gin, TANH)
                u = u_pool.tile([128, W], F16, tag=f"u{g}")
                nc.vector.scalar_tensor_tensor(
                    u[0:H], tau_t[0:H, 0:W], 1.0, cc[g][:], ADD, MULT)
                nc.vector.scalar_tensor_tensor(
                    u[H:128], tau_t[H:128, 0:W], 1.0,
                    tau_t[H:128, W:2 * W], ADD, MULT)
                pend_ist[g] = (u, tau_t, t)

            for chunk in range(n_chunks):
                xs = get_tile(chunk)
                for s in range(S):
                    t = chunk * S + s
                    gp = gpsum_pool.tile([128, 2 * Bc], F32, tag="gp")
                    # pipeline: rotate each group's ist/tanhc to just
                    # after the next group's phase1 (the last group's wraps
                    # into the following step).
                    for g in range(G):
                        phase1(g, gp, xs, s, t)
                        gprev = (g - 1) % G
                        if pend_ist[gprev] is not None:
                            emit_ist(gprev)
                            phase2(gprev)
                    if G == 1:
                        emit_ist(0)
                        phase2(0)
            for g in range(G):
                if pend_ist[g] is not None:
                    emit_ist(g)
                    phase2(g)

            fcp = gpsum_pool.tile([C_OUT, Bc], F32, tag="fcp", bufs=1)
            nc.tensor.matmul(fcp[:], fcw_sb[:], h_fin[:],
                             start=True, stop=True)
            logits = consts.tile([C_OUT, Bc], F32)
            nc.scalar.activation(logits[:], fcp[:], IDENT, bias=fcb_sb[:])
            nc.sync.dma_start(out=out[:], in_=logits[:])

    nc.compile()
    return nc


def _prep_weights(W_ih, W_hh, b_ih, b_hh, fc_W):
    """Fold sigmoid->tanh halving, h2 doubling, and biases into stationaries.

    Gate order in the reference weights is (i, f, g, o). FI half = [f; i]
    with scale 0.5; OG half = [o; g] with scales (0.5, 1.0). Stationary
    rows: 0:32 x-weights, 32 bias, 64:128 h-weights (extra 0.5 for h2=2h).
    """
    idx = {g: np.arange(k * H, (k + 1) * H) for k, g in enumerate("ifgo")}
    rows_FI = np.concatenate([idx["f"], idx["i"]])
    rows_OG = np.concatenate([idx["o"], idx["g"]])
    s_FI = np.full(128, 0.5, np.float32)
    s_OG = np.concatenate([np.full(64, 0.5, np.float32),
                           np.full(64, 1.0, np.float32)])
    b_sum = (b_ih + b_hh).astype(np.float32)

    sxh = np.zeros((2, R, 128), np.float32)
    for k, (rows, sc) in enumerate([(rows_FI, s_FI), (rows_OG, s_OG)]):
        sxh[k, 0:D] = (sc[:, None] * W_ih[rows]).T
        sxh[k, D] = sc * b_sum[rows]
        sxh[k, H2OFF:R] = (sc[:, None] * W_hh[rows] * 0.5).T
    ist = np.zeros((128, H), np.float32)
    ist[np.arange(H), np.arange(H)] = 0.5
    ist[np.arange(H) + H, np.arange(H)] = 0.5
    fcw = (0.5 * fc_W).T
    return (sxh.astype(np.float16), ist.astype(np.float16),
            fcw.astype(np.float16))


_NC_CACHE = {}


def kernel(x, W_ih, W_hh, b_ih, b_hh, fc_W, fc_b, _trace=False):
    x = np.asarray(x, np.float32)
    B, T, Dd = x.shape
    assert Dd == D
    Bc = B // N_CORES

    sxh, ist, fcw = _prep_weights(
        np.asarray(W_ih, np.float32), np.asarray(W_hh, np.float32),
        np.asarray(b_ih, np.float32), np.asarray(b_hh, np.float32),
        np.asarray(fc_W, np.float32))
    fcb = np.asarray(fc_b, np.float32).reshape(C_OUT, 1)

    key = (T, Bc)
    if key not in _NC_CACHE:
        _NC_CACHE[key] = build_lstm_nc(T, Bc)
    nc = _NC_CACHE[key]

    in_maps = []
    for core in range(N_CORES):
        xsl = x[core * Bc:(core + 1) * Bc]            # [Bc, T, D]
        xTc = np.empty((D + 1, T, Bc), np.float16)
        xTc[0:D] = xsl.transpose(2, 1, 0).astype(np.float16)
        xTc[D] = 1.0
        in_maps.append({
            "xT": xTc, "sxh": sxh, "istack": ist,
            "fc_w": fcw, "fc_b": fcb,
        })

    res = run_bass_kernel_spmd(nc, in_maps, core_ids=list(range(N_CORES)),
                               trace=_trace)
    outs = [r["out"] for r in res.results]            # each [C, Bc]
    logits = np.concatenate([o.T for o in outs], axis=0).astype(np.float32)
    if _trace:
        kernel.last_results = res
    return logits
